# revision 3
# baseline (speedup 1.0000x reference)
"""BitLinear 2-bit on 8 TRN2 cores — mixed bf16/fp8-DoubleRow K-split.

out = x @ W_deq^T + bias (absmax normalization cancels; scale=1).
W levels {±0.5, ±1.5} are exact in both bf16 and fp8e4 (e4m3).

Accuracy/speed split along the contraction: the first E=14 k-subtiles
(k < 1792) run in bf16 (x exact to bf16, ~0.1% error), the remaining
F=18 subtiles run in fp8 e4m3 with MatmulPerfMode.DoubleRow (2 k-subtiles
per PE instruction, ~1.4x bf16 throughput, x quantization ~2.5% rms rel).
Total rel error ≈ 0.025*sqrt(18/32) ≈ 0.019 < 2e-2 gate; measured DR
matmul = ~614 cyc vs bf16 521 → per-group 14*521+9*614 = 0.77x baseline.

Sharding: data-parallel over the 8192 rows; each core does [1024, 4096].
"""

import time

import numpy as np
import ml_dtypes

import concourse.mybir as mybir
from concourse import bacc
from concourse.tile import TileContext
from concourse.bass_utils import run_bass_kernel_spmd

N_CORES = 8
B, S, D_IN, D_OUT = 4, 2048, 4096, 4096
M_TOTAL = B * S              # 8192 rows
M = M_TOTAL // N_CORES       # 1024 rows per core
K = D_IN
N = D_OUT
P = 128
E = 14                       # bf16 k-subtiles
F = 18                       # fp8 k-subtiles (even, DoubleRow pairs)
KB = E * P                   # 1792
KF = F * P                   # 2304
NF = 512                     # psum free dim
NI = N // NF                 # 8 n-chunks
MI = M // P                  # 8 m-tiles

FP8 = mybir.dt.float8e4
BF16 = mybir.dt.bfloat16
F32 = mybir.dt.float32
DR = mybir.MatmulPerfMode.DoubleRow

# startup DMA piece sizes (k-subtiles); fp8 pieces even so DR pairs don't
# straddle a piece boundary
B_PIECES = [1, 1, 2, 2, 4, 4]
F_PIECES = [2, 4, 4, 4, 4]
assert sum(B_PIECES) == E and sum(F_PIECES) == F


def build(m=M, n=N):
    mi_n, ni_n = m // P, n // NF
    nc = bacc.Bacc()
    xTb = nc.declare_dram_parameter("xTb", [KB, m], BF16, isOutput=False)
    xTf = nc.declare_dram_parameter("xTf", [KF, m], FP8, isOutput=False)
    wTb = nc.declare_dram_parameter("wTb", [KB, n], BF16, isOutput=False)
    wTf = nc.declare_dram_parameter("wTf", [KF, n], FP8, isOutput=False)
    bias = nc.declare_dram_parameter("bias", [P, n], F32, isOutput=False)
    out = nc.declare_dram_parameter("out", [m, n], F32, isOutput=True)

    xb3 = xTb[:].rearrange("(a p) m -> p a m", p=P)   # [128, E, m]
    xf3 = xTf[:].rearrange("(a p) m -> p a m", p=P)   # [128, F, m]
    wb3 = wTb[:].rearrange("(a p) n -> p a n", p=P)   # [128, E, n]
    wf3 = wTf[:].rearrange("(a p) n -> p a n", p=P)   # [128, F, n]

    with TileContext(nc) as tc:
        with (
            tc.tile_pool(name="xpool", bufs=1) as xpool,
            tc.tile_pool(name="bpool", bufs=1) as bpool,
            tc.tile_pool(name="wbpool", bufs=2) as wbpool,
            tc.tile_pool(name="wfpool", bufs=2) as wfpool,
            tc.tile_pool(name="opool", bufs=6) as opool,
            tc.tile_pool(name="ppool", bufs=8, space="PSUM") as ppool,
        ):
            xb = xpool.tile([P, E, m], BF16, name="xb")
            xf = xpool.tile([P, F, m], FP8, name="xf")
            wb0 = wbpool.tile([P, E, NF], BF16, name="wb")
            wf0 = wfpool.tile([P, F, NF], FP8, name="wf")
            # interleave x and first-W-chunk loads in k-order pieces so the
            # first matmuls can start early; x on ACT DGE ring, W on SP ring
            pos = 0
            for cs in B_PIECES:
                sl = slice(pos, pos + cs)
                nc.scalar.dma_start(out=xb[:, sl, :], in_=xb3[:, sl, :])
                nc.sync.dma_start(out=wb0[:, sl, :], in_=wb3[:, sl, 0:NF])
                pos += cs
            pos = 0
            for cs in F_PIECES:
                sl = slice(pos, pos + cs)
                nc.scalar.dma_start(out=xf[:, sl, :], in_=xf3[:, sl, :])
                nc.sync.dma_start(out=wf0[:, sl, :], in_=wf3[:, sl, 0:NF])
                pos += cs
            bias_sb = bpool.tile([P, n], F32, name="bias_sb")
            nc.scalar.dma_start(out=bias_sb[:], in_=bias[:])

            # PE warmup on zeroed tiles (both dtypes) while startup DMA streams
            warm_l = bpool.tile([P, P], BF16, name="warm_l")
            warm_r = bpool.tile([P, NF], BF16, name="warm_r")
            warm_lf = bpool.tile([P, 2, P], FP8, name="warm_lf")
            warm_rf = bpool.tile([P, 2, NF], FP8, name="warm_rf")
            nc.vector.memset(warm_l[:], 0.0)
            nc.vector.memset(warm_r[:], 0.0)
            nc.vector.memset(warm_lf[:], 0.0)
            nc.vector.memset(warm_rf[:], 0.0)

            def epilogue(ps, mi, nsl, eng=None):
                # out-DMA on the SP ring alongside W loads; routing it to the
                # gpsimd DGE ring was tried and lost ~6us (slow descriptor
                # issue, ~750ns per DMA, delays output drain)
                ot = opool.tile([P, NF], F32, name="ot")
                nc.vector.tensor_add(out=ot[:], in0=ps[:], in1=bias_sb[:, nsl])
                (eng or nc.sync).dma_start(
                    out=out[mi * P:(mi + 1) * P, nsl], in_=ot[:])

            def group_matmuls(ps, wb, wf, mi, nslice=slice(0, NF)):
                msl = slice(mi * P, (mi + 1) * P)
                for kk in range(E):
                    nc.tensor.matmul(
                        ps[:], lhsT=xb[:, kk, msl], rhs=wb[:, kk, nslice],
                        start=(kk == 0), stop=False,
                    )
                for j in range(0, F, 2):
                    nc.tensor.matmul(
                        ps[:], lhsT=xf[:, j:j + 2, msl],
                        rhs=wf[:, j:j + 2, nslice],
                        start=False, stop=(j == F - 2), perf_mode=DR,
                    )

            wb, wf = wb0, wf0
            for ni in range(ni_n):
                nsl = slice(ni * NF, (ni + 1) * NF)
                wb_next = wf_next = None
                if ni + 1 < ni_n:
                    wb_next = wbpool.tile([P, E, NF], BF16, name="wb")
                    wf_next = wfpool.tile([P, F, NF], FP8, name="wf")
                if ni == 0:
                    # ki-piece-major across all 8 psum banks so the PE rides
                    # right behind the startup DMA stream
                    pss = [ppool.tile([P, NF], F32, name="ps") for _ in range(mi_n)]
                    for _ in range(8):
                        nc.tensor.matmul(
                            pss[mi_n - 1][:], lhsT=warm_l[:], rhs=warm_r[:],
                            start=True, stop=True,
                        )
                    for _ in range(4):
                        nc.tensor.matmul(
                            pss[mi_n - 1][:], lhsT=warm_lf[:], rhs=warm_rf[:],
                            start=True, stop=True, perf_mode=DR,
                        )
                    cpos = 0
                    for cs in B_PIECES:
                        for mi in range(mi_n):
                            msl = slice(mi * P, (mi + 1) * P)
                            for kk in range(cpos, cpos + cs):
                                nc.tensor.matmul(
                                    pss[mi][:], lhsT=xb[:, kk, msl],
                                    rhs=wb[:, kk, :],
                                    start=(kk == 0), stop=False,
                                )
                        cpos += cs
                    cpos = 0
                    for pi, cs in enumerate(F_PIECES):
                        for mi in range(mi_n):
                            msl = slice(mi * P, (mi + 1) * P)
                            for j in range(cpos, cpos + cs, 2):
                                nc.tensor.matmul(
                                    pss[mi][:], lhsT=xf[:, j:j + 2, msl],
                                    rhs=wf[:, j:j + 2, :],
                                    start=False, stop=(j == F - 2),
                                    perf_mode=DR,
                                )
                        cpos += cs
                        # issue ni=1 weight prefetch once startup stream done
                        # (issuing it earlier delays the ni=0 epilogue
                        # out-DMAs behind 2.9 MiB of W on the ring: net loss)
                        if pi == len(F_PIECES) - 1 and wb_next is not None:
                            for g in range(7):
                                sl = slice(2 * g, min(2 * g + 2, E))
                                nc.sync.dma_start(
                                    out=wb_next[:, sl, :],
                                    in_=wb3[:, sl, NF:2 * NF])
                            for g in range(9):
                                sl = slice(2 * g, 2 * g + 2)
                                nc.sync.dma_start(
                                    out=wf_next[:, sl, :],
                                    in_=wf3[:, sl, NF:2 * NF])
                    for mi in range(mi_n):
                        epilogue(pss[mi], mi, nsl)
                else:
                    # batch pairs of groups by PE mode: both groups' bf16
                    # matmuls, then both groups' DR matmuls — halves the
                    # bf16->DR transitions (each costs a ~190ns DR-entry
                    # stall) without the slot inflation seen with whole-chunk
                    # batching
                    for mp in range(mi_n // 2):
                        mis = (2 * mp, 2 * mp + 1)
                        pss = {mi: ppool.tile([P, NF], F32, name="ps")
                               for mi in mis}
                        for mi in mis:
                            msl = slice(mi * P, (mi + 1) * P)
                            for kk in range(E):
                                nc.tensor.matmul(
                                    pss[mi][:], lhsT=xb[:, kk, msl],
                                    rhs=wb[:, kk, :],
                                    start=(kk == 0), stop=False,
                                )
                            # spread next-chunk weight DMA over the phase:
                            # slots 0-2 load the bf16 subtiles (5+5+4), slots
                            # 3-7 the fp8 subtiles (4+4+4+4+2)
                            if wb_next is not None:
                                nxt = slice((ni + 1) * NF, (ni + 2) * NF)
                                if mi < 3:
                                    sl = slice(5 * mi, min(5 * mi + 5, E))
                                    nc.sync.dma_start(
                                        out=wb_next[:, sl, :],
                                        in_=wb3[:, sl, nxt])
                                else:
                                    g = mi - 3
                                    sl = slice(4 * g, min(4 * g + 4, F))
                                    nc.sync.dma_start(
                                        out=wf_next[:, sl, :],
                                        in_=wf3[:, sl, nxt])
                        for mi in mis:
                            msl = slice(mi * P, (mi + 1) * P)
                            final_group = ni == ni_n - 1 and mi == mi_n - 1
                            for j in range(0, F - 2 if final_group else F, 2):
                                nc.tensor.matmul(
                                    pss[mi][:], lhsT=xf[:, j:j + 2, msl],
                                    rhs=wf[:, j:j + 2, :],
                                    start=False, stop=(j == F - 2),
                                    perf_mode=DR,
                                )
                            if not final_group:
                                epilogue(pss[mi], mi, nsl)
                                continue
                            # very last group: run the final DR pair as two
                            # half-width matmuls with a half epilogue after
                            # each, so the kernel-tail drain is one short
                            # 256-wide DVE+DMA chain instead of a full one
                            hf = NF // 2
                            for half in range(2):
                                hs = slice(half * hf, (half + 1) * hf)
                                nc.tensor.matmul(
                                    pss[mi][:, hs],
                                    lhsT=xf[:, F - 2:F, msl],
                                    rhs=wf[:, F - 2:F, hs],
                                    start=False, stop=True, perf_mode=DR,
                                )
                                hsl = slice(ni * NF + half * hf,
                                            ni * NF + (half + 1) * hf)
                                ot = opool.tile([P, hf], F32, name="ot")
                                nc.vector.tensor_add(
                                    out=ot[:], in0=pss[mi][:, hs],
                                    in1=bias_sb[:, hsl])
                                nc.sync.dma_start(
                                    out=out[mi * P:(mi + 1) * P, hsl],
                                    in_=ot[:])
                wb, wf = wb_next, wf_next
    nc.finalize()
    return nc


_NC = None


def _get_nc():
    global _NC
    if _NC is None:
        _NC = build()
    return _NC


def make_in_maps(x, weight_2bit, weight_scale, bias):
    x = np.asarray(x)
    codes = np.asarray(weight_2bit)
    ws = np.float32(np.asarray(weight_scale).reshape(-1)[0])
    b = np.asarray(bias).astype(np.float32)

    w_f = (codes.astype(np.float32) - np.float32(1.5)) * ws      # [N, K]
    wT = np.ascontiguousarray(w_f.T)                             # [K, N]
    wTb = np.ascontiguousarray(wT[:KB].astype(ml_dtypes.bfloat16))
    wTf = np.ascontiguousarray(wT[KB:].astype(ml_dtypes.float8_e4m3))
    bias_rep = np.ascontiguousarray(np.broadcast_to(b, (P, N)))

    x2 = x.reshape(M_TOTAL, K)
    in_maps = []
    for c in range(N_CORES):
        xc = x2[c * M:(c + 1) * M]
        xTbc = np.ascontiguousarray(xc[:, :KB].T.astype(ml_dtypes.bfloat16))
        xTfc = np.ascontiguousarray(xc[:, KB:].T.astype(ml_dtypes.float8_e4m3))
        in_maps.append(
            {"xTb": xTbc, "xTf": xTfc, "wTb": wTb, "wTf": wTf,
             "bias": bias_rep})
    return in_maps


def run(in_maps, trace=False, **kw):
    # The axon-tunneled devices occasionally fail a fresh process's first
    # execution with NRT_EXEC_UNIT_UNRECOVERABLE; an identical retry succeeds.
    last = None
    for attempt in range(4):
        try:
            return run_bass_kernel_spmd(
                _get_nc(), in_maps, list(range(N_CORES)), trace=trace, **kw
            )
        except Exception as e:
            last = e
            msg = str(e)
            if "UNAVAILABLE" in msg or "unrecoverable" in msg.lower():
                try:
                    import jax

                    jax.clear_caches()
                    import jax.extend.backend

                    jax.extend.backend.clear_backends()
                except Exception:
                    pass
                time.sleep(15 * (attempt + 1))
                continue
            raise
    raise last


def kernel(x, weight_2bit, weight_scale, bias):
    res = run(make_in_maps(x, weight_2bit, weight_scale, bias))
    out = np.concatenate([r["out"] for r in res.results], axis=0)
    return np.ascontiguousarray(out.reshape(B, S, N))
